# revision 1
# baseline (speedup 1.0000x reference)
"""Trainium2 Bass kernel for nn_C_Aggregation_24807731101830.

Patch-embed conv (stride 16 = kernel 16) + sequential Gauss-Seidel-like
index-update scan over a flattened 34x34 grid, batch-sharded over 8 cores.

Per core (2 local batches):
  - conv as matmul: out[c, (b,q)] = sum_k wT[k, c] * patches[k, (b,q)], k=768
  - 34x34 grid border = bias-only; interior scattered from PSUM with bias add
  - the scan: row-recurrence y[j] = (1/8) y[j-1] + (1/8)(3-tap prev row + 4-tap
    orig) solved with the DVE TensorTensorScan instruction, one op per row i,
    12 independent (batch, channel-group) segments per op via A=0 reset cols.
"""
import sys
import types
import numpy as np

import concourse.mybir as mybir
from concourse import bass, tile
from concourse.bass_utils import run_bass_kernel_spmd
from contextlib import ExitStack

F32 = mybir.dt.float32
F32R = mybir.dt.float32r
AOP = mybir.AluOpType
IDENT = mybir.ActivationFunctionType.Identity

N_CORES = 8
B_LOC = 2            # batches per core
CG = 6               # channel groups of 128
NBG = B_LOC * CG     # 12 scan lanes-groups
Q34 = 1156           # 34*34
QF = NBG * Q34       # buf free size per partition

LAST_EXEC_NS = None


def _install_ntff_hook():
    try:
        import trn_agent_boot.trn_boot as tb
        mod = types.ModuleType("antenv.axon_hooks")
        holder = [None]
        mod.set_axon_ntff_profile_hook = lambda h: holder.__setitem__(0, h)
        mod.get_axon_ntff_profile_hook = lambda: holder[0]
        sys.modules["antenv.axon_hooks"] = mod
        import antenv
        antenv.axon_hooks = mod
        mod.set_axon_ntff_profile_hook(
            tb._ntff_profile_via_ctypes('/opt/axon/libaxon_pjrt.so'))
        return True
    except Exception:
        return False


def _split_sp_multiwaits(nc):
    """walrus for gen3 rejects >1 sync-wait on several instruction structs
    (TPB_CTRL, S3_LW, ...); hoist extra waits onto single-wait NOPs placed
    just before, on the same engine queue (semantically equivalent)."""
    cnt = 0
    for f in nc.m.functions:
        for blk in f.blocks:
            insts = blk.instructions
            i = 0
            while i < len(insts):
                inst = insts[i]
                si = getattr(inst, 'sync_info', None)
                if (getattr(inst, 'engine', None) is not None
                        and si is not None and si.on_wait and len(si.on_wait) > 1):
                    waits = list(si.on_wait)
                    new = []
                    for w in waits[:-1]:
                        nop = mybir.InstNoOp(name=f"mwfix-{inst.name}-{cnt}",
                                             ins=[], outs=[])
                        cnt += 1
                        nop.engine = inst.engine
                        nop.sync_info = mybir.SyncInfo(on_wait=[w], on_update=[])
                        new.append(nop)
                    inst.sync_info = mybir.SyncInfo(
                        on_wait=[waits[-1]], on_update=list(si.on_update or []))
                    insts[i:i] = new
                    i += len(new)
                i += 1
    return cnt


def _build():
    nc = bass.Bass("TRN2", target_bir_lowering=False)
    xP_d = nc.declare_dram_parameter("xP", [768, B_LOC, 1024], F32R, isOutput=False)
    wT_d = nc.declare_dram_parameter("wT", [768, 768], F32R, isOutput=False)
    bias_d = nc.declare_dram_parameter("bias", [768], F32, isOutput=False)
    xf_d = nc.declare_dram_parameter("xf", [B_LOC, 768, Q34], F32, isOutput=True)

    with tile.TileContext(nc) as tc, ExitStack() as ctx:
        sb = ctx.enter_context(tc.tile_pool(name="sb", bufs=1))
        sc = ctx.enter_context(tc.tile_pool(name="sc", bufs=3))
        ps = ctx.enter_context(tc.tile_pool(name="ps", bufs=4, space="PSUM"))

        # ---- loads ----
        wt = sb.tile([128, 6, 768], F32R, tag="wt")
        wTr = wT_d.rearrange("(a p) c -> p a c", p=128)
        for a in range(6):
            nc.sync.dma_start(wt[:, a:a + 1, :], wTr[:, a:a + 1, :])
        xpt = sb.tile([128, 6, B_LOC * 1024], F32R, tag="xpt")
        xPr = xP_d.rearrange("(a p) b q -> p a (b q)", p=128)
        for a in range(6):
            nc.sync.dma_start(xpt[:, a:a + 1, :], xPr[:, a:a + 1, :])
        biast = sb.tile([128, 6], F32, tag="bias")
        nc.sync.dma_start(biast[:], bias_d.rearrange("(a p) -> p a", p=128))

        # ---- constants ----
        amask = sb.tile([128, NBG * 33], F32, tag="amask")
        nc.vector.memset(amask[:], 0.125)
        am3 = amask[:].rearrange("p (g c) -> p g c", g=NBG)
        nc.vector.memset(am3[:, :, 0:1], 0.0)
        nc.vector.memset(am3[:, :, 32:33], 0.0)
        zt = sb.tile([128, 64], F32, tag="zt")
        nc.vector.memset(zt[:], 0.0)

        # ---- output buffer: f = bg*1156 + q34 ----
        buf = sb.tile([128, QF], F32, tag="buf")
        buf3 = buf[:].rearrange("p (bg q) -> p bg q", bg=NBG)
        buf4 = buf[:].rearrange("p (bg gi gj) -> p bg gi gj", bg=NBG, gi=34)

        # ---- borders = bias (emitted FIRST so ACT does them before
        #      scatters: the scan chain depends on them via S0) ----
        for b in range(B_LOC):
            for m in range(CG):
                bg = b * CG + m
                bcol = biast[:, m:m + 1]
                nc.scalar.activation(buf3[:, bg, 0:35], zt[:, 0:35],
                                     IDENT, bias=bcol)
                prs = buf3[:, bg:bg + 1, 67:67 + 34 * 31].rearrange(
                    "p o (r t) -> p (o r) t", t=34)[:, :, 0:2]
                zp = zt[:, 0:62].rearrange("p (r t) -> p r t", t=2)
                nc.scalar.activation(prs, zp, IDENT, bias=bcol)
                nc.scalar.activation(buf3[:, bg, 1121:1156], zt[:, 0:35],
                                     IDENT, bias=bcol)

        # S_i layout [128, 12 segs x 33]: cols 0..32 = buf[32i .. 32i+32]
        s_prev_box = [None]
        s_prev_box[0] = sc.tile([128, NBG * 33], F32, tag="S", name="s0", bufs=4)
        nc.scalar.mul(
            s_prev_box[0][:].rearrange("p (g c) -> p g c", g=NBG),
            buf3[:, :, 0:33], 1.0)

        RB = 6          # rows per P band
        NB = 30 // RB

        def band_view(base, nrows):
            # [p, r, g, j<31] with strides [., 32, 1156, 1] from buf
            return buf3[:, :, base:base + 32 * nrows].rearrange(
                "p g (r t) -> p r g t", t=32)[:, :, :, 0:31]

        pbpool = ctx.enter_context(tc.tile_pool(name="pb", bufs=3))
        ptp = ctx.enter_context(tc.tile_pool(name="ptp", bufs=1))
        pb_scaled = {}

        def emit_band(bnd):
            i0 = RB * bnd + 1
            base = 32 * i0 + 2
            pb = pbpool.tile([128, RB * NBG * 31], F32, tag="PB",
                             name=f"pb_{bnd}")
            pb4 = pb[:].rearrange("p (r g j) -> p r g j", r=RB, g=NBG)
            nc.vector.tensor_tensor(pb4, band_view(base, RB),
                                    band_view(base + 30, RB), AOP.add)
            tmp = ptp.tile([128, RB * NBG * 31], F32, tag="PTMP",
                           name=f"ptmp_{bnd}")
            tmp4 = tmp[:].rearrange("p (r g j) -> p r g j", r=RB, g=NBG)
            nc.vector.tensor_tensor(tmp4, band_view(base + 31, RB),
                                    band_view(base + 32, RB), AOP.add)
            nc.vector.tensor_tensor(pb[:], pb[:], tmp[:], AOP.add)
            nc.scalar.mul(pb[:], pb[:], 0.125)   # P/8 in place on ACT
            pb_scaled[bnd] = pb

        def emit_row(i):
            qi = 32 * i
            bnd, r = divmod(i - 1, RB)
            psc = pb_scaled[bnd][:].rearrange(
                "p (r g j) -> p r g j", r=RB, g=NBG)[:, r, :, :]
            bt = sc.tile([128, NBG * 33], F32, tag="B", name=f"bt_{i}", bufs=6)
            b3 = bt[:].rearrange("p (g c) -> p g c", g=NBG)
            nc.scalar.mul(b3[:, :, 0:1], buf3[:, :, qi:qi + 1], 1.0)
            nc.scalar.mul(b3[:, :, 32:33], buf3[:, :, qi + 32:qi + 33], 1.0)
            s3p = s_prev_box[0][:].rearrange("p (g c) -> p g c", g=NBG)
            u1 = sc.tile([128, NBG * 31], F32, tag="u1", name=f"u1_{i}")
            u1v = u1[:].rearrange("p (g c) -> p g c", g=NBG)
            nc.vector.tensor_tensor(u1v, s3p[:, :, 0:31], s3p[:, :, 1:32],
                                    AOP.add)
            u2 = sc.tile([128, NBG * 31], F32, tag="u2", name=f"u2_{i}")
            u2v = u2[:].rearrange("p (g c) -> p g c", g=NBG)
            nc.vector.tensor_tensor(u2v, u1v, s3p[:, :, 2:33], AOP.add)
            # B[1:32] = u2/8 + P/8  (the per-row 1/8 of the reference update)
            nc.vector.scalar_tensor_tensor(
                b3[:, :, 1:32], u2v, 0.125, psc, AOP.mult, AOP.add)
            s_cur = sc.tile([128, NBG * 33], F32, tag="S", name=f"s_{i}", bufs=4)
            nc.vector.tensor_tensor_scan(s_cur[:], amask[:], bt[:], 0.0,
                                         AOP.mult, AOP.add)
            nc.scalar.mul(
                buf3[:, :, qi + 1:qi + 32],
                s_cur[:].rearrange("p (g c) -> p g c", g=NBG)[:, :, 1:32], 1.0)
            s_prev_box[0] = s_cur

        def emit_conv_pair(pair):
            for m in range(CG):
                pts = {}
                for nq in pair:
                    pts[nq] = ps.tile([128, 512], F32, tag="ps",
                                      name=f"pt_{m}_{nq}")
                for a in range(6):
                    for nq in pair:
                        nc.tensor.matmul(
                            pts[nq][:],
                            lhsT=wt[:, a, 128 * m:128 * (m + 1)],
                            rhs=xpt[:, a, 512 * nq:512 * (nq + 1)],
                            start=(a == 0), stop=(a == 5))
                for nq in pair:
                    b, gih = divmod(nq, 2)
                    dst = buf4[:, b * CG + m, 1 + 16 * gih:17 + 16 * gih, 1:33]
                    nc.scalar.activation(
                        dst, pts[nq][:].rearrange("p (gi gj) -> p gi gj", gi=16),
                        IDENT, bias=biast[:, m:m + 1])

        def emit_conv_slice(b, o):
            # n-slice = batch b, gi-octet o (grid rows 8o..8o+7), N=256
            off = b * 1024 + o * 256
            for m in range(CG):
                pt = ps.tile([128, 256], F32, tag="ps", name=f"pt_{b}_{o}_{m}")
                for a in range(6):
                    nc.tensor.matmul(
                        pt[:],
                        lhsT=wt[:, a, 128 * m:128 * (m + 1)],
                        rhs=xpt[:, a, off:off + 256],
                        start=(a == 0), stop=(a == 5))
                dst = buf4[:, b * CG + m, 1 + 8 * o:9 + 8 * o, 1:33]
                nc.scalar.activation(
                    dst, pt[:].rearrange("p (gi gj) -> p gi gj", gi=8),
                    IDENT, bias=biast[:, m:m + 1])

        # interleave conv gi-octets with the scan chain: after octet o the
        # chain rows whose reads fall inside gi<=8o+7 are emitted, so the
        # DVE chain overlaps nearly all PE/ACT conv work
        sched = {0: (  [0],    range(1, 6)),
                 1: (  [1],    range(6, 13)),
                 2: (  [2],    range(13, 19)),
                 3: ([3, 4],   range(19, 31))}
        for o in range(4):
            emit_conv_slice(0, o)
            emit_conv_slice(1, o)
            bands, rows = sched[o]
            for bnd in bands:
                emit_band(bnd)
            for i in rows:
                emit_row(i)
                if i == 16:
                    bv = buf[:].rearrange("p (b g q) -> p b g q",
                                          b=B_LOC, g=CG)
                    for g in range(CG):
                        dst = xf_d[:, 128 * g:128 * (g + 1), 0:545].rearrange(
                            "b p q -> p b q")
                        nc.sync.dma_start(dst, bv[:, :, g, 0:545])

        # ---- dump the rest ----
        for g in range(CG):
            src = buf[:].rearrange("p (b g q) -> p b g q", b=B_LOC, g=CG)[:, :, g, 545:]
            dst = xf_d[:, 128 * g:128 * (g + 1), 545:].rearrange("b p q -> p b q")
            nc.sync.dma_start(dst, src)

    _split_sp_multiwaits(nc)
    return nc


_NC = None


def kernel(x: np.ndarray, w: np.ndarray, b: np.ndarray) -> np.ndarray:
    global _NC, LAST_EXEC_NS
    B, C, H, _ = x.shape          # 16, 3, 512, 512
    assert (B, C, H) == (16, 3, 512)

    # host layout prep (sharding + im2col layout): patches[k, b, q]
    # k = c_in*256 + py*16 + px ; q = gi*32 + gj
    xp = x.reshape(B, 3, 32, 16, 32, 16)               # b c gi py gj px
    xp = np.ascontiguousarray(xp.transpose(1, 3, 5, 0, 2, 4))  # c py px b gi gj
    xp = xp.reshape(768, B, 1024)
    wT = np.ascontiguousarray(w.reshape(768, 768).T)   # [k, c]
    b = np.ascontiguousarray(b, dtype=np.float32)

    if _NC is None:
        _NC = _build()

    trace = _install_ntff_hook()
    in_maps = [{"xP": np.ascontiguousarray(xp[:, 2 * r:2 * r + 2, :]),
                "wT": wT, "bias": b} for r in range(N_CORES)]
    try:
        res = run_bass_kernel_spmd(_NC, in_maps, core_ids=list(range(N_CORES)),
                                   trace=trace)
    except Exception:
        if not trace:
            raise
        res = run_bass_kernel_spmd(_NC, in_maps, core_ids=list(range(N_CORES)),
                                   trace=False)
    LAST_EXEC_NS = res.exec_time_ns
    globals()['LAST_RESULT'] = res

    xf = np.concatenate([res.results[r]["xf"] for r in range(N_CORES)], axis=0)
    out = xf.reshape(B, 3, 544, 544)[:, :, 16:528, 16:528]
    return np.ascontiguousarray(out)



# revision 4
# speedup vs baseline: 1.3549x; 1.3549x over previous
"""Trainium2 Bass kernel for nn_C_Aggregation_24807731101830.

Patch-embed conv (stride 16 = kernel 16) + sequential Gauss-Seidel-like
index-update scan over a flattened 34x34 grid, batch-sharded over 8 cores.

v2 (fp16): all on-chip data in fp16 (DVE 2x modes, half DMA traffic),
conv matmuls on fp16 inputs (full PE rate), and the row scan in
"multiply-form": state = (d0 + state) * M with M = 1/0.125/0 mask, which
folds the *1/8 of the reference update into the scan so the band
precompute needs no scale pass and the per-row critical path is three
2x tensor_tensor ops plus the scan.

Per core (2 local batches):
  - conv as matmul: out[c, (b,q)] = sum_k wT[k, c] * patches[k, (b,q)], k=768
  - 34x34 grid border = bias-only; interior scattered from PSUM with bias add
  - scan row i (stride-32 rows of the flat grid): d0 = 3-tap(prev row) +
    4-tap(orig)  [4-tap precomputed in octet-aligned band pieces], then
    y = scan(d0; y <- (d0 + y)*0.125), written back to buf cols 1..31.
Output is fp16 in DRAM; the host upcasts to fp32 (tolerance is 2e-2).
"""
import sys
import types
import numpy as np

import concourse.mybir as mybir
from concourse import bass, tile
from concourse.bass_utils import run_bass_kernel_spmd
from contextlib import ExitStack

F32 = mybir.dt.float32
F16 = mybir.dt.float16
AOP = mybir.AluOpType
IDENT = mybir.ActivationFunctionType.Identity

N_CORES = 8
B_LOC = 2            # batches per core
CG = 6               # channel groups of 128
NBG = B_LOC * CG     # 12 scan lane-groups
Q34 = 1156           # 34*34
QF = NBG * Q34       # buf free size per partition

LAST_EXEC_NS = None


def _install_ntff_hook():
    try:
        import trn_agent_boot.trn_boot as tb
        mod = types.ModuleType("antenv.axon_hooks")
        holder = [None]
        mod.set_axon_ntff_profile_hook = lambda h: holder.__setitem__(0, h)
        mod.get_axon_ntff_profile_hook = lambda: holder[0]
        sys.modules["antenv.axon_hooks"] = mod
        import antenv
        antenv.axon_hooks = mod
        mod.set_axon_ntff_profile_hook(
            tb._ntff_profile_via_ctypes('/opt/axon/libaxon_pjrt.so'))
        return True
    except Exception:
        return False


def _split_sp_multiwaits(nc):
    """walrus for gen3 rejects >1 sync-wait on several instruction structs
    (TPB_CTRL, S3_LW, ...); hoist extra waits onto single-wait NOPs placed
    just before, on the same engine queue (semantically equivalent)."""
    cnt = 0
    for f in nc.m.functions:
        for blk in f.blocks:
            insts = blk.instructions
            i = 0
            while i < len(insts):
                inst = insts[i]
                si = getattr(inst, 'sync_info', None)
                if (getattr(inst, 'engine', None) is not None
                        and si is not None and si.on_wait and len(si.on_wait) > 1):
                    waits = list(si.on_wait)
                    new = []
                    for w in waits[:-1]:
                        nop = mybir.InstNoOp(name=f"mwfix-{inst.name}-{cnt}",
                                             ins=[], outs=[])
                        cnt += 1
                        nop.engine = inst.engine
                        nop.sync_info = mybir.SyncInfo(on_wait=[w], on_update=[])
                        new.append(nop)
                    inst.sync_info = mybir.SyncInfo(
                        on_wait=[waits[-1]], on_update=list(si.on_update or []))
                    insts[i:i] = new
                    i += len(new)
                i += 1
    return cnt


# band pieces: rows of the scan that become runnable after conv octet o
# (octet o scatters grid rows 8o+1..8o+8, i.e. flat < 272*o + 305; row i
# reads taps up to flat 32i+65)
PIECES = [(1, 7), (8, 16), (17, 24), (25, 30)]   # inclusive row ranges
# output DMA chunks: flat [lo, hi) ready after the given row
# (row r+1 writes from flat 32(r+1)+1, so after row r everything below
# 32(r+1)+1 is final)
CHUNKS = [(0, 512, 15), (512, 737, 22), (737, 1156, 30)]


def _build():
    nc = bass.Bass("TRN2", target_bir_lowering=False)
    xP_d = nc.declare_dram_parameter("xP", [768, B_LOC, 1024], F16, isOutput=False)
    wT_d = nc.declare_dram_parameter("wT", [768, 768], F16, isOutput=False)
    bias_d = nc.declare_dram_parameter("bias", [768], F32, isOutput=False)
    xf_d = nc.declare_dram_parameter("xf", [B_LOC, 768, Q34], F16, isOutput=True)

    with tile.TileContext(nc) as tc, ExitStack() as ctx:
        sb = ctx.enter_context(tc.tile_pool(name="sb", bufs=1))
        sc = ctx.enter_context(tc.tile_pool(name="sc", bufs=3))
        ps = ctx.enter_context(tc.tile_pool(name="ps", bufs=4, space="PSUM"))
        pbpool = ctx.enter_context(tc.tile_pool(name="pb", bufs=2))

        # ---- loads: weights first, then the x quarters needed earliest ----
        wt = sb.tile([128, 6, 768], F16, tag="wt")
        wTr = wT_d.rearrange("(a p) c -> p a c", p=128)
        for a in range(6):
            nc.sync.dma_start(wt[:, a:a + 1, :], wTr[:, a:a + 1, :])
        biast = sb.tile([128, 6], F32, tag="bias")
        nc.sync.dma_start(biast[:], bias_d.rearrange("(a p) -> p a", p=128))
        xpt = sb.tile([128, 6, B_LOC * 1024], F16, tag="xpt")
        xPr = xP_d.rearrange("(a p) b q -> p a b q", p=128)
        xpt4 = xpt[:].rearrange("p a (b q) -> p a b q", b=B_LOC)
        for quarter in range(4):
            q0, q1 = 256 * quarter, 256 * (quarter + 1)
            for a in range(6):
                for b in range(B_LOC):
                    nc.sync.dma_start(xpt4[:, a:a + 1, b:b + 1, q0:q1],
                                      xPr[:, a:a + 1, b:b + 1, q0:q1])

        # ---- constants ----
        # multiply-form scan mask: per 33-col segment [1, 0.125*31, 0]
        mmask = sb.tile([128, NBG * 33], F16, tag="mmask")
        nc.vector.memset(mmask[:], 0.125)
        mm3 = mmask[:].rearrange("p (g c) -> p g c", g=NBG)
        nc.vector.memset(mm3[:, :, 0:1], 1.0)
        nc.vector.memset(mm3[:, :, 32:33], 0.0)
        zt = sb.tile([128, 64], F16, tag="zt")
        nc.vector.memset(zt[:], 0.0)

        # ---- output buffer: f = bg*1156 + flat ----
        buf = sb.tile([128, QF], F16, tag="buf")
        buf3 = buf[:].rearrange("p (bg q) -> p bg q", bg=NBG)
        buf4 = buf[:].rearrange("p (bg gi gj) -> p bg gi gj", bg=NBG, gi=34)

        # ---- borders = bias (emitted FIRST so ACT does them before
        #      scatters: the scan chain depends on them) ----
        for b in range(B_LOC):
            for m in range(CG):
                bg = b * CG + m
                bcol = biast[:, m:m + 1]
                nc.scalar.activation(buf3[:, bg, 0:35], zt[:, 0:35],
                                     IDENT, bias=bcol)
                prs = buf3[:, bg:bg + 1, 67:67 + 34 * 31].rearrange(
                    "p o (r t) -> p (o r) t", t=34)[:, :, 0:2]
                zp = zt[:, 0:62].rearrange("p (r t) -> p r t", t=2)
                nc.scalar.activation(prs, zp, IDENT, bias=bcol)
                nc.scalar.activation(buf3[:, bg, 1121:1156], zt[:, 0:35],
                                     IDENT, bias=bcol)

        def band_view(base, nrows):
            # [p, r, g, j<31] with strides [., 32, 1156, 1] from buf
            return buf3[:, :, base:base + 32 * nrows].rearrange(
                "p g (r t) -> p r g t", t=32)[:, :, :, 0:31]

        pb_piece = {}

        def emit_band(piece):
            """4-tap of orig (UNSCALED: the scan's M-mask provides the /8)
            for rows i0..i1, plus the col-31 fix (the 3-tap's rightmost
            tap at j=31 is orig(flat 32i) which the zeroed s_prev col 32
            cannot supply)."""
            i0, i1 = PIECES[piece]
            nr = i1 - i0 + 1
            base = 32 * i0 + 2
            pb = pbpool.tile([128, nr * NBG * 31], F16, tag="PB",
                             name=f"pb_{piece}")
            pb4 = pb[:].rearrange("p (r g j) -> p r g j", r=nr, g=NBG)
            # gpsimd takes one of the three passes off the DVE
            nc.gpsimd.tensor_tensor(pb4, band_view(base, nr),
                                    band_view(base + 30, nr), AOP.add)
            tmp = pbpool.tile([128, nr * NBG * 31], F16, tag="PTMP",
                              name=f"ptmp_{piece}")
            tmp4 = tmp[:].rearrange("p (r g j) -> p r g j", r=nr, g=NBG)
            nc.vector.tensor_tensor(tmp4, band_view(base + 31, nr),
                                    band_view(base + 32, nr), AOP.add)
            nc.vector.tensor_tensor(pb[:], pb[:], tmp[:], AOP.add)
            # col-31 fix: += buf[flat 32i] for each row i (skip row 1: its
            # 3-tap reads buf directly and already sees flat 32)
            f0 = i0 if i0 > 1 else 2
            if f0 <= i1:
                nfix = i1 - f0 + 1
                fix_dst = pb4[:, f0 - i0:, :, 30:31]
                fix_src = buf3[:, :, 32 * f0:32 * f0 + 32 * nfix].rearrange(
                    "p g (r t) -> p r g t", t=32)[:, :, :, 0:1]
                nc.vector.scalar_tensor_tensor(
                    fix_dst, fix_src, 1.0, fix_dst, AOP.mult, AOP.add)
            pb_piece[piece] = (pb, i0)

        s_prev_box = [None]

        def emit_row(i):
            qi = 32 * i
            piece = next(p for p, (a, b) in enumerate(PIECES) if a <= i <= b)
            pb, i0 = pb_piece[piece]
            pbr = pb[:].rearrange("p (r g j) -> p r g j",
                                  r=PIECES[piece][1] - i0 + 1,
                                  g=NBG)[:, i - i0, :, :]
            if s_prev_box[0] is None:
                sp = buf3[:, :, 0:33]          # row 0 = orig, uncorrupted
            else:
                sp = s_prev_box[0][:].rearrange("p (g c) -> p g c", g=NBG)
            # d0[j] = sp[j-1] + sp[j] + sp[j+1] + P4[j]   (j = 1..31)
            ua = sc.tile([128, NBG * 31], F16, tag="ua", name=f"ua_{i}")
            uav = ua[:].rearrange("p (g c) -> p g c", g=NBG)
            nc.vector.tensor_tensor(uav, sp[:, :, 0:31], sp[:, :, 2:33],
                                    AOP.add)
            ub = sc.tile([128, NBG * 31], F16, tag="ub", name=f"ub_{i}")
            ubv = ub[:].rearrange("p (g c) -> p g c", g=NBG)
            nc.vector.tensor_tensor(ubv, sp[:, :, 1:32], pbr, AOP.add)
            d0 = sc.tile([128, NBG * 33], F16, tag="d0", name=f"d0_{i}",
                         bufs=4)
            d3 = d0[:].rearrange("p (g c) -> p g c", g=NBG)
            nc.vector.tensor_tensor(d3[:, :, 1:32], uav, ubv, AOP.add)
            # col 0 seeds the segment (M=1); col 32 is killed by M=0 but
            # must be finite (uninitialized SBUF can be NaN as fp16)
            nc.scalar.mul(d3[:, :, 0:1], buf3[:, :, qi:qi + 1], 1.0)
            nc.scalar.mul(d3[:, :, 32:33], zt[:, 0:12].rearrange(
                "p (g c) -> p g c", g=NBG), 1.0)
            s_cur = sc.tile([128, NBG * 33], F16, tag="S", name=f"s_{i}",
                            bufs=4)
            nc.vector.tensor_tensor_scan(s_cur[:], d0[:], mmask[:], 0.0,
                                         AOP.add, AOP.mult)
            # write back cols 1..31 only (col 32 is the zeroed reset slot;
            # col 0 is unchanged orig)
            nc.vector.tensor_scalar(
                buf3[:, :, qi + 1:qi + 32],
                s_cur[:].rearrange("p (g c) -> p g c", g=NBG)[:, :, 1:32],
                1.0, None, op0=AOP.mult)
            s_prev_box[0] = s_cur

        def emit_conv_slice(b, o):
            # n-slice = batch b, gi-octet o (grid rows 8o+1..8o+8), N=256
            off = b * 1024 + o * 256
            for m in range(CG):
                pt = ps.tile([128, 256], F32, tag="ps", name=f"pt_{b}_{o}_{m}")
                for a in range(6):
                    nc.tensor.matmul(
                        pt[:],
                        lhsT=wt[:, a, 128 * m:128 * (m + 1)],
                        rhs=xpt[:, a, off:off + 256],
                        start=(a == 0), stop=(a == 5))
                dst = buf4[:, b * CG + m, 1 + 8 * o:9 + 8 * o, 1:33]
                nc.scalar.activation(
                    dst, pt[:].rearrange("p (gi gj) -> p gi gj", gi=8),
                    IDENT, bias=biast[:, m:m + 1])

        def emit_out_chunk(lo, hi):
            bv = buf[:].rearrange("p (b g q) -> p b g q", b=B_LOC, g=CG)
            for g in range(CG):
                dst = xf_d[:, 128 * g:128 * (g + 1), lo:hi].rearrange(
                    "b p q -> p b q")
                nc.sync.dma_start(dst, bv[:, :, g, lo:hi])

        chunk_after = {r: (lo, hi) for lo, hi, r in CHUNKS}
        for o in range(4):
            emit_conv_slice(0, o)
            emit_conv_slice(1, o)
            emit_band(o)
            i0, i1 = PIECES[o]
            for i in range(i0, i1 + 1):
                emit_row(i)
                if i in chunk_after:
                    emit_out_chunk(*chunk_after[i])

    _split_sp_multiwaits(nc)
    return nc


_NC = None


def kernel(x: np.ndarray, w: np.ndarray, b: np.ndarray) -> np.ndarray:
    global _NC, LAST_EXEC_NS
    B, C, H, _ = x.shape          # 16, 3, 512, 512
    assert (B, C, H) == (16, 3, 512)

    # host layout prep (sharding + im2col layout): patches[k, b, q]
    # k = c_in*256 + py*16 + px ; q = gi*32 + gj
    xp = x.reshape(B, 3, 32, 16, 32, 16)               # b c gi py gj px
    xp = np.ascontiguousarray(
        xp.transpose(1, 3, 5, 0, 2, 4)).reshape(768, B, 1024)
    xp = xp.astype(np.float16)
    wT = np.ascontiguousarray(w.reshape(768, 768).T).astype(np.float16)
    b = np.ascontiguousarray(b, dtype=np.float32)

    if _NC is None:
        _NC = _build()

    trace = _install_ntff_hook()
    in_maps = [{"xP": np.ascontiguousarray(xp[:, 2 * r:2 * r + 2, :]),
                "wT": wT, "bias": b} for r in range(N_CORES)]
    try:
        res = run_bass_kernel_spmd(_NC, in_maps, core_ids=list(range(N_CORES)),
                                   trace=trace)
    except Exception:
        if not trace:
            raise
        res = run_bass_kernel_spmd(_NC, in_maps, core_ids=list(range(N_CORES)),
                                   trace=False)
    LAST_EXEC_NS = res.exec_time_ns
    globals()['LAST_RESULT'] = res

    xf = np.concatenate([res.results[r]["xf"] for r in range(N_CORES)], axis=0)
    out = xf.reshape(B, 3, 544, 544)[:, :, 16:528, 16:528]
    return np.ascontiguousarray(out.astype(np.float32))
